# revision 1
# baseline (speedup 1.0000x reference)
"""Trainium2 Bass kernel for one dense transformer block (RMSNorm -> causal
RoPE attention -> residual -> RMSNorm -> GELU MLP -> residual).

Sharding across 8 NeuronCores: 2 batch-groups (data parallel over B=2) x 4
ranks. Within a group: tensor-parallel over heads for QKV+attention, 4-rank
ReduceScatter reshards the out_proj partial sums to sequence-parallel, then
each rank runs the MLP on its own 512-token shard with full weights. The host
reassembles the per-core sequence shards.

Layout notes:
- All activations feeding matmuls are float32r (full-rate PE, ~FP22).
- Q/K weight columns are de-interleaved per head on the host (even rotary
  dims -> partitions 0..63, odd -> 64..127) so RoPE is contiguous DVE math;
  q.k dot products are invariant to the shared permutation.
- RMSNorm scales fold into the RoPE tables (q,k) / a per-token PSUM-copy
  scale (v); g_in/g_ff fold into the weights on the host.
- Softmax skips the max-subtraction (scores are ~N(0,1); exp cannot
  overflow) so denominators come from ones-vector matmuls on the PE.
"""

import os
import sys

import numpy as np

for _p in ("/root/.axon_site/_ro/trn_rl_repo", "/opt/trn_rl_repo"):
    if os.path.isdir(_p) and _p not in sys.path:
        sys.path.append(_p)

import concourse.bass as bass  # noqa: E402
import concourse.mybir as mybir  # noqa: E402
import concourse.tile as tile  # noqa: E402
from concourse import bacc  # noqa: E402
from concourse.bass_utils import run_bass_kernel_spmd  # noqa: E402

F32 = mybir.dt.float32
F32R = mybir.dt.float32r
F16 = mybir.dt.float16
AF = mybir.ActivationFunctionType

G = 4  # ranks per batch-group
NCORES = 8
DH = 128  # head dim (= partition width)
EPS = 1e-6
ROPE_BASE = 10000.0


def _bcast_ap(ap, parts):
    """Partition-broadcast view of a [1, N] AP -> [parts, N]."""
    a2 = ap.opt()
    return bass.AP(tensor=a2.tensor, offset=a2.offset, ap=[[0, parts]] + list(a2.ap))


def build_nc(S, D, H, FF, gelu=None):
    gelu = gelu if gelu is not None else AF.Gelu_apprx_tanh
    HC = H // G  # heads per core
    SC = S // 4  # chunk length == sequence shard length
    ND = D // 128
    NQK = 2 * HC  # q then k feature blocks (128 wide)
    NV = HC * DH
    NKBC = SC // 128  # 128-token k-blocks per chunk
    NFF = FF // 128
    ISQ = float(1.0 / np.sqrt(DH))

    nc = bacc.Bacc("TRN2", target_bir_lowering=False, debug=False, num_devices=NCORES)

    xT_in = nc.dram_tensor("xT", [4, 128, ND, SC], F32R, kind="ExternalInput")
    xrT_in = nc.dram_tensor("xrT", [128, ND, SC], F32, kind="ExternalInput")
    wqk_in = nc.dram_tensor("wqk", [NQK, 128, ND, 128], F32R, kind="ExternalInput")
    wv_in = nc.dram_tensor("wv", [128, ND, NV], F32R, kind="ExternalInput")
    wout_in = nc.dram_tensor("wout", [128, HC, D], F32R, kind="ExternalInput")
    wfc_in = nc.dram_tensor("wfc", [NFF, 128, ND, 128], F32R, kind="ExternalInput")
    wproj_in = nc.dram_tensor("wproj", [ND, 128, NFF, 128], F32R, kind="ExternalInput")
    ctab_in = nc.dram_tensor("ctab", [64, S], F32, kind="ExternalInput")
    stab_in = nc.dram_tensor("stab", [64, S], F32, kind="ExternalInput")
    mask_in = nc.dram_tensor("masks", [128, NKBC, SC], F32R, kind="ExternalInput")
    ident_in = nc.dram_tensor("ident", [128, 128], F32R, kind="ExternalInput")
    ones_in = nc.dram_tensor("ones", [128, 1], F32R, kind="ExternalInput")
    out_ext = nc.dram_tensor("out", [ND, 128, SC], F32, kind="ExternalOutput")

    with tile.TileContext(nc) as tc:
        with (
            tc.tile_pool(name="const", bufs=1) as constp,
            tc.tile_pool(name="dram", bufs=1, space="DRAM") as dramp,
        ):
            q_dram = dramp.tile([4, 128, HC, SC], F32R)
            k_dram = dramp.tile([128, HC, S], F32R)
            v_dram = dramp.tile([128, S // 128, NV], F32R)
            r_dram = dramp.tile([1, S], F32)
            r2_dram = dramp.tile([1, SC], F32)
            den_dram = dramp.tile([1, HC * SC], F32)
            NMG = 4  # RS pipeline chunks over dout blocks
            MGW = ND // NMG
            RSDT = mybir.dt.float16
            rs_in_g = [dramp.tile([G, MGW, 128, SC], RSDT, name=f"rsi{i}",
                                  tag=f"rsi{i}") for i in range(NMG)]
            rs_out_g = [dramp.tile([MGW, 128, SC], RSDT, name=f"rso{i}",
                                   tag=f"rso{i}") for i in range(NMG)]

            ones_sb = constp.tile([128, 1], F32R)
            nc.sync.dma_start(ones_sb[:], ones_in[:])
            eps_sb = constp.tile([1, 1], F32)
            nc.vector.memset(eps_sb[:], EPS)
            ident_sb = constp.tile([128, 128], F32R)
            nc.sync.dma_start(ident_sb[:], ident_in[:])

            # ================= phase A: stats + QKV + RoPE =================
            with (
                tc.tile_pool(name="a_w", bufs=1) as awp,
                tc.tile_pool(name="a_stream", bufs=3) as astr,
                tc.tile_pool(name="a_x", bufs=2) as axp,
                tc.tile_pool(name="a_work", bufs=2) as awk,
                tc.tile_pool(name="a_ps", bufs=3, space="PSUM") as apsum,
                tc.tile_pool(name="a_ps1", bufs=1, space="PSUM") as apsum1,
            ):
                wv_sb = awp.tile([128, ND, NV], F32R)
                nc.sync.dma_start(wv_sb[:], wv_in[:])
                ctab_sb = awp.tile([64, S], F32)
                stab_sb = awp.tile([64, S], F32)
                nc.sync.dma_start(ctab_sb[:], ctab_in[:])
                nc.sync.dma_start(stab_sb[:], stab_in[:])

                for c in range(4):
                    csl = slice(c * SC, (c + 1) * SC)
                    x_sb = axp.tile([128, ND, SC], F32R, tag="xchunk")
                    nc.sync.dma_start(x_sb[:], xT_in[c])

                    # rms stats: ss[tok] = sum_d x^2
                    ps_ss = apsum1.tile([1, SC], F32, tag="ss")
                    for d in range(ND):
                        xsq = awk.tile([128, SC], F32R, tag="xsq")
                        nc.scalar.square(xsq[:], x_sb[:, d, :])
                        nc.tensor.matmul(
                            ps_ss[:], ones_sb[:], xsq[:],
                            start=(d == 0), stop=(d == ND - 1),
                        )
                    r_sb = awk.tile([1, SC], F32, tag="rsb")
                    nc.scalar.activation(
                        r_sb[:], ps_ss[:], AF.Sqrt, bias=eps_sb[:], scale=1.0 / D
                    )
                    nc.vector.reciprocal(r_sb[:], r_sb[:])
                    nc.sync.dma_start(r_dram[0:1, csl], r_sb[:])

                    r64 = awk.tile([64, SC], F32, tag="r64")
                    nc.sync.dma_start(r64[:], _bcast_ap(r_dram[0:1, csl], 64))
                    r_tok = awk.tile([128, NKBC], F32, tag="rtok")
                    flat = r_dram[0:1, csl]
                    nc.sync.dma_start(
                        r_tok[:],
                        bass.AP(tensor=flat.tensor, offset=flat.offset,
                                ap=[[1, 128], [128, NKBC]]),
                    )

                    ctr = awk.tile([64, SC], F32, tag="ctr")
                    srt = awk.tile([64, SC], F32, tag="srt")
                    nc.vector.tensor_mul(ctr[:], ctab_sb[:, csl], r64[:])
                    nc.vector.tensor_mul(srt[:], stab_sb[:, csl], r64[:])

                    # q/k matmuls + rope (features de-interleaved on host:
                    # rotary-even dims on partitions 0:64, odd on 64:128)
                    for m in range(NQK):
                        wm = astr.tile([128, ND, 128], F32R, tag="wqk")
                        nc.sync.dma_start(wm[:], wqk_in[m])
                        ps_qk = apsum.tile([128, SC], F32, tag="qk")
                        for d in range(ND):
                            nc.tensor.matmul(
                                ps_qk[:], wm[:, d, :], x_sb[:, d, :],
                                start=(d == 0), stop=(d == ND - 1),
                            )
                        ro = awk.tile([128, SC], F32R, tag="ro")
                        t1 = awk.tile([64, SC], F32, tag="t1")
                        t2 = awk.tile([64, SC], F32, tag="t2")
                        nc.vector.tensor_mul(t1[:], ps_qk[0:64, :], ctr[:])
                        nc.vector.tensor_mul(t2[:], ps_qk[64:128, :], srt[:])
                        nc.vector.tensor_sub(ro[0:64, :], t1[:], t2[:])
                        nc.vector.tensor_mul(t1[:], ps_qk[64:128, :], ctr[:])
                        nc.vector.tensor_mul(t2[:], ps_qk[0:64, :], srt[:])
                        nc.vector.tensor_add(ro[64:128, :], t1[:], t2[:])
                        if m < HC:
                            nc.sync.dma_start(q_dram[c][:, m, :], ro[:])
                        else:
                            nc.sync.dma_start(k_dram[:, m - HC, csl], ro[:])

                    # v matmuls (token-major), fold rms scale into PSUM copy
                    for sb in range(NKBC):
                        ps_v = apsum.tile([128, NV], F32, tag="v")
                        for d in range(ND):
                            nc.tensor.matmul(
                                ps_v[:], x_sb[:, d, sb * 128:(sb + 1) * 128],
                                wv_sb[:, d, :],
                                start=(d == 0), stop=(d == ND - 1),
                            )
                        vst = awk.tile([128, NV], F32R, tag="vst")
                        nc.scalar.activation(
                            vst[:], ps_v[:], AF.Copy, scale=r_tok[:, sb:sb + 1]
                        )
                        nc.sync.dma_start(v_dram[:, c * NKBC + sb, :], vst[:])

            # ================= phases B+C under shared avT =================
            # ========== phase B: attention + fused out_proj partials ==========
            with (
                tc.tile_pool(name="b_kv", bufs=1) as bkv,
                tc.tile_pool(name="b_q", bufs=2) as bq,
                tc.tile_pool(name="b_work", bufs=2) as bwk,
                tc.tile_pool(name="b_pt", bufs=6) as bpt,
                tc.tile_pool(name="b_av", bufs=2) as bav,
                tc.tile_pool(name="b_den", bufs=1) as bdn,
                tc.tile_pool(name="b_ps", bufs=3, space="PSUM") as bps,
                tc.tile_pool(name="b_ps_acc", bufs=2, space="PSUM") as bpsa,
                tc.tile_pool(name="b_ps_op", bufs=2, space="PSUM") as bpso,
            ):
                krT = bkv.tile([128, HC, S], F32R)
                vtok = bkv.tile([128, S // 128, NV], F32R)
                masks = bkv.tile([128, NKBC, SC], F32R)
                wout_sb = bkv.tile([128, HC, D], F32R)
                nc.sync.dma_start(wout_sb[:], wout_in[:])
                for c in range(4):
                    csl = slice(c * SC, (c + 1) * SC)
                    bsl = slice(c * NKBC, (c + 1) * NKBC)
                    nc.sync.dma_start(krT[:, :, csl], k_dram[:, :, csl])
                    nc.sync.dma_start(vtok[:, bsl, :], v_dram[:, bsl, :])
                nc.sync.dma_start(masks[:], mask_in[:])

                for c in range(4):
                    csl = slice(c * SC, (c + 1) * SC)
                    qc = bq.tile([128, HC, SC], F32R, tag="qc")
                    nc.sync.dma_start(qc[:], q_dram[c])
                    nkb = (c + 1) * NKBC
                    den_c = bdn.tile([1, HC * SC], F32, tag="den")
                    avc = bav.tile([128, HC, SC], F32R, tag="avc")
                    for h in range(HC):
                        ps_av = bpsa.tile([128, SC], F32, tag="av")
                        ps_den = bpsa.tile([1, SC], F32, tag="dn", bufs=1)
                        for w0 in range(0, nkb, 4):
                            wave = range(w0, min(w0 + 4, nkb))
                            pts = {}
                            for kb in wave:
                                ps_sc = bps.tile([128, SC], F32, tag="sc")
                                o = kb - c * NKBC
                                if o >= 0:
                                    nc.tensor.matmul(
                                        ps_sc[:], ident_sb[:], masks[:, o, :],
                                        start=True, stop=False,
                                    )
                                nc.tensor.matmul(
                                    ps_sc[:],
                                    krT[:, h, kb * 128:(kb + 1) * 128],
                                    qc[:, h, :],
                                    start=(o < 0), stop=True,
                                )
                                pT = bpt.tile([128, SC], F32R, tag="pt")
                                nc.scalar.activation(
                                    pT[:], ps_sc[:], AF.Exp, scale=ISQ
                                )
                                pts[kb] = pT
                            for kb in wave:
                                nc.tensor.matmul(
                                    ps_av[:],
                                    vtok[:, kb, h * DH:(h + 1) * DH],
                                    pts[kb][:],
                                    start=(kb == 0), stop=(kb == nkb - 1),
                                )
                                nc.tensor.matmul(
                                    ps_den[:], ones_sb[:], pts[kb][:],
                                    start=(kb == 0), stop=(kb == nkb - 1),
                                )
                        nc.vector.reciprocal(
                            den_c[:, h * SC:(h + 1) * SC], ps_den[:]
                        )
                        nc.scalar.copy(avc[:, h, :], ps_av[:])
                    nc.sync.dma_start(den_dram[:], den_c[:])
                    denb = bwk.tile([128, HC * SC], F32, tag="denb")
                    nc.sync.dma_start(denb[:], _bcast_ap(den_dram[:], 128))
                    for h in range(HC):
                        nc.vector.tensor_mul(
                            avc[:, h, :], avc[:, h, :],
                            denb[:, h * SC:(h + 1) * SC],
                        )
                    # fused out_proj partials for this q-chunk
                    for mg in range(NMG):
                        for ml in range(MGW):
                            m = mg * MGW + ml
                            ps_op = bpso.tile([128, SC], F32, tag="op")
                            for fb in range(HC):
                                nc.tensor.matmul(
                                    ps_op[:],
                                    wout_sb[:, fb, m * 128:(m + 1) * 128],
                                    avc[:, fb, :],
                                    start=(fb == 0), stop=(fb == HC - 1),
                                )
                            ost = bwk.tile([128, SC], RSDT, tag="ost")
                            nc.scalar.copy(ost[:], ps_op[:])
                            nc.sync.dma_start(rs_in_g[mg][c, ml], ost[:])

            for mg in range(NMG):
                nc.gpsimd.collective_compute(
                    "ReduceScatter",
                    mybir.AluOpType.add,
                    replica_groups=[[0, 1, 2, 3], [4, 5, 6, 7]],
                    ins=[rs_in_g[mg][:].opt()],
                    outs=[rs_out_g[mg][:].opt()],
                )

            # ============ phase D: residual + RMSNorm2 + MLP ==============
            with (
                tc.tile_pool(name="d_res", bufs=1) as dres,
                tc.tile_pool(name="d_stream", bufs=3) as dstr,
                tc.tile_pool(name="d_work", bufs=2) as dwk,
                tc.tile_pool(name="d_ps", bufs=2, space="PSUM") as dps,
                tc.tile_pool(name="d_ps1", bufs=1, space="PSUM") as dps1,
            ):
                out1 = dres.tile([128, ND, SC], F32)
                ps_ss2 = dps1.tile([1, SC], F32, tag="ss2")
                for d in range(ND):
                    rsb = dwk.tile([128, SC], RSDT, tag="rsb2")
                    nc.sync.dma_start(rsb[:], rs_out_g[d // MGW][d % MGW])
                    xr = dwk.tile([128, SC], F32, tag="xr")
                    nc.sync.dma_start(xr[:], xrT_in[:, d, :])
                    nc.vector.tensor_add(out1[:, d, :], rsb[:], xr[:])
                    xsq = dwk.tile([128, SC], F32R, tag="xsq2")
                    nc.scalar.square(xsq[:], out1[:, d, :])
                    nc.tensor.matmul(
                        ps_ss2[:], ones_sb[:], xsq[:],
                        start=(d == 0), stop=(d == ND - 1),
                    )
                r2_sb = dwk.tile([1, SC], F32, tag="r2")
                nc.scalar.activation(
                    r2_sb[:], ps_ss2[:], AF.Sqrt, bias=eps_sb[:], scale=1.0 / D
                )
                nc.vector.reciprocal(r2_sb[:], r2_sb[:])
                nc.sync.dma_start(r2_dram[:], r2_sb[:])
                r2b = dres.tile([128, SC], F32)
                nc.sync.dma_start(r2b[:], _bcast_ap(r2_dram[:], 128))

                h2T = dres.tile([128, ND, SC], F32R)
                for d in range(ND):
                    nc.vector.tensor_mul(h2T[:, d, :], out1[:, d, :], r2b[:])

                gT = dres.tile([128, NFF, SC], F32R)
                for m in range(NFF):
                    wm = dstr.tile([128, ND, 128], F32R, tag="wfc")
                    nc.sync.dma_start(wm[:], wfc_in[m])
                    ps_fc = dps.tile([128, SC], F32, tag="fc")
                    for d in range(ND):
                        nc.tensor.matmul(
                            ps_fc[:], wm[:, d, :], h2T[:, d, :],
                            start=(d == 0), stop=(d == ND - 1),
                        )
                    nc.scalar.activation(gT[:, m, :], ps_fc[:], gelu)

                for m in range(ND):
                    wm = dstr.tile([128, NFF, 128], F32R, tag="wproj", bufs=2)
                    nc.sync.dma_start(wm[:], wproj_in[m])
                    ps_pr = dps.tile([128, SC], F32, tag="pr")
                    for k in range(NFF):
                        nc.tensor.matmul(
                            ps_pr[:], wm[:, k, :], gT[:, k, :],
                            start=(k == 0), stop=(k == NFF - 1),
                        )
                    o2 = dwk.tile([128, SC], F32, tag="o2")
                    nc.vector.tensor_add(o2[:], ps_pr[:], out1[:, m, :])
                    nc.sync.dma_start(out_ext[m], o2[:])

    nc.compile()
    return nc


def _deinterleave(w):
    """Reorder head-dim columns: evens then odds (per 128-wide head)."""
    Din, Dout = w.shape
    nh = Dout // DH
    w4 = w.reshape(Din, nh, DH // 2, 2)
    return np.concatenate([w4[..., 0], w4[..., 1]], axis=2).reshape(Din, Dout)


def prep_inputs(x, w_qkv, w_out, w_fc, w_proj, g_in, g_ff, S, D, H, FF):
    HC = H // G
    SC = S // 4
    ND = D // 128
    NQK = 2 * HC
    NV = HC * DH
    NKBC = SC // 128
    NFF = FF // 128

    x = np.asarray(x, np.float32)
    w_qkv = np.asarray(w_qkv, np.float32)
    w_out = np.asarray(w_out, np.float32)
    w_fc = np.asarray(w_fc, np.float32)
    w_proj = np.asarray(w_proj, np.float32)
    g_in = np.asarray(g_in, np.float32)
    g_ff = np.asarray(g_ff, np.float32)

    wq = w_qkv * g_in[:, None]

    half = DH // 2
    invf = 1.0 / (ROPE_BASE ** (2.0 * np.arange(half, dtype=np.float64) / DH))
    ang = np.arange(S, dtype=np.float64)[:, None] * invf[None, :]
    ctab = np.ascontiguousarray(np.cos(ang).T.astype(np.float32))
    stab = np.ascontiguousarray(np.sin(ang).T.astype(np.float32))

    masks = np.zeros((NKBC, 128, SC), np.float32)
    pp = np.arange(128)[:, None]
    qq = np.arange(SC)[None, :]
    for o in range(NKBC):
        masks[o] = np.where(o * 128 + pp <= qq, 0.0, -60000.0).astype(np.float32)
    masks = np.ascontiguousarray(masks.transpose(1, 0, 2))
    ident = np.eye(128, dtype=np.float32)

    wfc = w_fc * g_ff[:, None]
    wfc_r = np.ascontiguousarray(wfc.reshape(ND, 128, NFF, 128).transpose(2, 1, 0, 3))
    wproj_r = np.ascontiguousarray(
        w_proj.reshape(NFF, 128, ND, 128).transpose(2, 1, 0, 3)
    )
    ones = np.ones((128, 1), np.float32)

    in_maps = []
    for core in range(NCORES):
        b, t = core // G, core % G
        xb = x[b]
        xT = np.ascontiguousarray(xb.T)
        xT_r = np.ascontiguousarray(xT.reshape(ND, 128, 4, SC).transpose(2, 1, 0, 3))
        xrT = np.ascontiguousarray(
            xb[t * SC:(t + 1) * SC, :].T.reshape(ND, 128, SC).transpose(1, 0, 2)
        )
        qcols = _deinterleave(wq[:, t * NV:(t + 1) * NV])
        kcols = _deinterleave(wq[:, D + t * NV:D + (t + 1) * NV])
        vcols = wq[:, 2 * D + t * NV:2 * D + (t + 1) * NV]
        wqk_core = np.ascontiguousarray(
            np.concatenate([qcols, kcols], axis=1)
            .reshape(ND, 128, NQK, 128).transpose(2, 1, 0, 3)
        )
        wv_core = np.ascontiguousarray(vcols.reshape(ND, 128, NV).transpose(1, 0, 2))
        wout_core = np.ascontiguousarray(
            w_out[t * NV:(t + 1) * NV, :].reshape(HC, 128, D).transpose(1, 0, 2)
        )
        in_maps.append({
            "xT": xT_r, "xrT": xrT, "wqk": wqk_core, "wv": wv_core,
            "wout": wout_core, "wfc": wfc_r, "wproj": wproj_r,
            "ctab": ctab, "stab": stab, "masks": masks, "ones": ones,
            "ident": ident,
        })
    return in_maps


def assemble(results, S, D):
    SC = S // 4
    y = np.zeros((2, S, D), np.float32)
    for core in range(NCORES):
        b, t = core // G, core % G
        o = results[core]["out"]
        y[b, t * SC:(t + 1) * SC, :] = o.reshape(D, SC).T
    return y


_CACHE = {}


def run(inputs, S, D, H, FF, trace=False, **kw):
    key = (S, D, H, FF)
    if key not in _CACHE:
        _CACHE[key] = build_nc(S, D, H, FF)
    nc = _CACHE[key]
    in_maps = prep_inputs(
        inputs["x"], inputs["w_qkv"], inputs["w_out"], inputs["w_fc"],
        inputs["w_proj"], inputs["g_in"], inputs["g_ff"], S, D, H, FF,
    )
    res = run_bass_kernel_spmd(nc, in_maps, list(range(NCORES)), trace=trace, **kw)
    return assemble(res.results, S, D), res


def kernel(**inputs):
    y, _ = run(inputs, S=2048, D=2048, H=16, FF=4096)
    return y.astype(np.float32)



# revision 2
# speedup vs baseline: 1.1668x; 1.1668x over previous
"""Trainium2 Bass kernel for one dense transformer block (RMSNorm -> causal
RoPE attention -> residual -> RMSNorm -> GELU MLP -> residual).

Sharding across 8 NeuronCores: 2 batch-groups (data parallel over B=2) x 4
ranks. Within a group: tensor-parallel over heads for QKV+attention. The
normalized per-head attention outputs (fp16, S x D/4 per rank) are exchanged
with four small per-chunk AllGathers issued inside the attention phase (so
they overlap compute); each rank then reads its own 512-token chunk of the
gathered buffer via a partition_id-driven dynamic DMA offset and runs
out_proj + RMSNorm + MLP + residuals for that chunk with full weights.

Layout/precision notes:
- All matmul operands are fp16 (f32 PSUM accumulation); fp16 LDWEIGHTS is
  ~4x faster than f32r, which lifts the back-to-back matmul issue rate.
- Q/K weight columns are de-interleaved per head on the host (even rotary
  dims -> partitions 0..63, odd -> 64..127) so RoPE is contiguous DVE math.
- RMSNorm scales fold into the RoPE tables (q,k) / a per-token PSUM-copy
  scale (v); g_in/g_ff fold into the weights on the host.
- Q/K/V live in SBUF between phases (no DRAM round-trip).
- Softmax skips the max-subtraction (scores are ~N(0,1); exp cannot
  overflow) so denominators come from ones-vector matmuls on the PE.
"""

import os
import sys

import numpy as np

for _p in ("/root/.axon_site/_ro/trn_rl_repo", "/opt/trn_rl_repo"):
    if os.path.isdir(_p) and _p not in sys.path:
        sys.path.append(_p)

import concourse.bass as bass  # noqa: E402
import concourse.mybir as mybir  # noqa: E402
import concourse.tile as tile  # noqa: E402
from concourse import bacc  # noqa: E402
from concourse.bass_utils import run_bass_kernel_spmd  # noqa: E402

F32 = mybir.dt.float32
F16 = mybir.dt.float16
AF = mybir.ActivationFunctionType

G = 4  # ranks per batch-group
NCORES = 8
DH = 128  # head dim (= partition width)
EPS = 1e-6
ROPE_BASE = 10000.0
CGRP = [[0, 1, 2, 3], [4, 5, 6, 7]]


def _bcast_ap(ap, parts):
    """Partition-broadcast view of a [1, N] AP -> [parts, N]."""
    a2 = ap.opt()
    return bass.AP(tensor=a2.tensor, offset=a2.offset, ap=[[0, parts]] + list(a2.ap))


def build_nc(S, D, H, FF, gelu=None):
    gelu = gelu if gelu is not None else AF.Gelu_apprx_tanh
    HC = H // G  # heads per core
    SC = S // 4  # chunk length == sequence shard length
    ND = D // 128
    NQK = 2 * HC  # q then k feature blocks (128 wide)
    NV = HC * DH
    NKBC = SC // 128  # 128-token k-blocks per chunk
    NB = S // 128
    NFF = FF // 128
    ISQ = float(1.0 / np.sqrt(DH))

    nc = bacc.Bacc("TRN2", target_bir_lowering=False, debug=False, num_devices=NCORES)

    xT_in = nc.dram_tensor("xT", [4, 128, ND, SC], F16, kind="ExternalInput")
    xr_in = nc.dram_tensor("xr", [128, ND, SC], F16, kind="ExternalInput")
    wqk_in = nc.dram_tensor("wqk", [NQK, 128, ND, 128], F16, kind="ExternalInput")
    wv_in = nc.dram_tensor("wv", [128, ND, NV], F16, kind="ExternalInput")
    wout_in = nc.dram_tensor("wout", [128, ND, D], F16, kind="ExternalInput")
    wfc_in = nc.dram_tensor("wfc", [NFF, 128, ND, 128], F16, kind="ExternalInput")
    wproj_in = nc.dram_tensor("wproj", [ND, 128, NFF, 128], F16, kind="ExternalInput")
    ctab_in = nc.dram_tensor("ctab", [64, S], F32, kind="ExternalInput")
    stab_in = nc.dram_tensor("stab", [64, S], F32, kind="ExternalInput")
    mask_in = nc.dram_tensor("masks", [128, NKBC, SC], F16, kind="ExternalInput")
    ident_in = nc.dram_tensor("ident", [128, 128], F16, kind="ExternalInput")
    ones_in = nc.dram_tensor("ones", [128, 1], F16, kind="ExternalInput")
    out_ext = nc.dram_tensor("out", [ND, 128, SC], F32, kind="ExternalOutput")

    with tile.TileContext(nc) as tc:
        with (
            tc.tile_pool(name="const", bufs=1) as constp,
            tc.tile_pool(name="dram", bufs=1, space="DRAM") as dramp,
            tc.tile_pool(name="kvq", bufs=1) as kvqp,
        ):
            r_dram = dramp.tile([1, S], F32)
            den_dram = dramp.tile([1, HC * SC], F32)
            r2_dram = dramp.tile([1, SC], F32)
            ag_in = [dramp.tile([128, HC, SC], F16, name=f"agi{c}", tag=f"agi{c}")
                     for c in range(4)]
            ag_all = dramp.tile([4, G, 128, HC, SC], F16)

            ones_sb = constp.tile([128, 1], F16)
            nc.sync.dma_start(ones_sb[:], ones_in[:])
            ident_sb = constp.tile([128, 128], F16)
            nc.sync.dma_start(ident_sb[:], ident_in[:])
            eps_sb = constp.tile([1, 1], F32)
            nc.vector.memset(eps_sb[:], EPS)

            k_sb = kvqp.tile([128, HC, S], F16)
            q_sb = kvqp.tile([128, HC, S], F16)
            v_sb = kvqp.tile([128, NB, NV], F16)

            # ================= phase A: stats + QKV + RoPE =================
            with (
                tc.tile_pool(name="a_w", bufs=1) as awp,
                tc.tile_pool(name="a_stream", bufs=3) as astr,
                tc.tile_pool(name="a_x", bufs=2) as axp,
                tc.tile_pool(name="a_work", bufs=2) as awk,
                tc.tile_pool(name="a_ps", bufs=3, space="PSUM") as apsum,
                tc.tile_pool(name="a_ps1", bufs=1, space="PSUM") as apsum1,
            ):
                wv_sb = awp.tile([128, ND, NV], F16)
                ctab_sb = awp.tile([64, S], F32)
                stab_sb = awp.tile([64, S], F32)

                for c in range(4):
                    csl = slice(c * SC, (c + 1) * SC)
                    x_sb = axp.tile([128, ND, SC], F16, tag="xchunk")
                    for dd in range(0, ND, 4):
                        nc.sync.dma_start(
                            x_sb[:, dd:dd + 4, :], xT_in[c][:, dd:dd + 4, :]
                        )
                    if c == 0:
                        nc.sync.dma_start(ctab_sb[:], ctab_in[:])
                        nc.sync.dma_start(stab_sb[:], stab_in[:])
                        nc.sync.dma_start(wv_sb[:], wv_in[:])

                    # rms stats: ss[tok] = sum_d x^2
                    ps_ss = apsum1.tile([1, SC], F32, tag="ss")
                    for d in range(ND):
                        xsq = awk.tile([128, SC], F16, tag="xsq")
                        nc.scalar.square(xsq[:], x_sb[:, d, :])
                        nc.tensor.matmul(
                            ps_ss[:], ones_sb[:], xsq[:],
                            start=(d == 0), stop=(d == ND - 1),
                        )
                    r_sb = awk.tile([1, SC], F32, tag="rsb")
                    nc.scalar.activation(
                        r_sb[:], ps_ss[:], AF.Sqrt, bias=eps_sb[:], scale=1.0 / D
                    )
                    nc.vector.reciprocal(r_sb[:], r_sb[:])
                    nc.sync.dma_start(r_dram[0:1, csl], r_sb[:])

                    r64 = awk.tile([64, SC], F32, tag="r64")
                    nc.sync.dma_start(r64[:], _bcast_ap(r_dram[0:1, csl], 64))
                    r_tok = awk.tile([128, NKBC], F32, tag="rtok")
                    flat = r_dram[0:1, csl]
                    nc.sync.dma_start(
                        r_tok[:],
                        bass.AP(tensor=flat.tensor, offset=flat.offset,
                                ap=[[1, 128], [128, NKBC]]),
                    )

                    ctr = awk.tile([64, SC], F32, tag="ctr")
                    srt = awk.tile([64, SC], F32, tag="srt")
                    nc.vector.tensor_mul(ctr[:], ctab_sb[:, csl], r64[:])
                    nc.vector.tensor_mul(srt[:], stab_sb[:, csl], r64[:])

                    # q/k matmuls + rope (features de-interleaved on host:
                    # rotary-even dims on partitions 0:64, odd on 64:128)
                    for m in range(NQK):
                        wm = astr.tile([128, ND, 128], F16, tag="wqk")
                        nc.sync.dma_start(wm[:], wqk_in[m])
                        ps_qk = apsum.tile([128, SC], F32, tag="qk")
                        for d in range(ND):
                            nc.tensor.matmul(
                                ps_qk[:], wm[:, d, :], x_sb[:, d, :],
                                start=(d == 0), stop=(d == ND - 1),
                            )
                        dst = q_sb if m < HC else k_sb
                        hh = m if m < HC else m - HC
                        t1 = awk.tile([64, SC], F32, tag="t1")
                        t2 = awk.tile([64, SC], F32, tag="t2")
                        nc.vector.tensor_mul(t1[:], ps_qk[0:64, :], ctr[:])
                        nc.vector.tensor_mul(t2[:], ps_qk[64:128, :], srt[:])
                        nc.vector.tensor_sub(dst[0:64, hh, csl], t1[:], t2[:])
                        nc.vector.tensor_mul(t1[:], ps_qk[64:128, :], ctr[:])
                        nc.vector.tensor_mul(t2[:], ps_qk[0:64, :], srt[:])
                        nc.vector.tensor_add(dst[64:128, hh, csl], t1[:], t2[:])

                    # v matmuls (token-major), fold rms scale into PSUM copy
                    for sb_i in range(NKBC):
                        ps_v = apsum.tile([128, NV], F32, tag="v")
                        for d in range(ND):
                            nc.tensor.matmul(
                                ps_v[:], x_sb[:, d, sb_i * 128:(sb_i + 1) * 128],
                                wv_sb[:, d, :],
                                start=(d == 0), stop=(d == ND - 1),
                            )
                        nc.scalar.activation(
                            v_sb[:, c * NKBC + sb_i, :], ps_v[:], AF.Copy,
                            scale=r_tok[:, sb_i:sb_i + 1],
                        )

            # ====== residual tile lives from phase C into phase D ======
            with tc.tile_pool(name="res", bufs=1) as resp:
                out1 = resp.tile([128, ND, SC], F32)
                r2b = resp.tile([128, SC], F32)

                with tc.tile_pool(name="wout", bufs=1) as woutp:
                    wout_sb = woutp.tile([128, ND, D], F16)

                    # ========== phase B: attention + per-chunk AllGather ==========
                    with (
                        tc.tile_pool(name="b_m", bufs=1) as bmp,
                        tc.tile_pool(name="b_work", bufs=2) as bwk,
                        tc.tile_pool(name="b_pt", bufs=6) as bpt,
                        tc.tile_pool(name="b_av", bufs=2) as bav,
                        tc.tile_pool(name="b_ps", bufs=3, space="PSUM") as bps,
                        tc.tile_pool(name="b_ps_acc", bufs=2, space="PSUM") as bpsa,
                    ):
                        masks = bmp.tile([128, NKBC, SC], F16)
                        nc.sync.dma_start(masks[:], mask_in[:])
                        nc.sync.dma_start(wout_sb[:], wout_in[:])

                        for c in range(4):
                            csl = slice(c * SC, (c + 1) * SC)
                            nkb = (c + 1) * NKBC
                            avf = bav.tile([128, HC, SC], F16, tag="avf")
                            for h in range(HC):
                                ps_av = bpsa.tile([128, SC], F32, tag="av")
                                ps_den = bpsa.tile([1, SC], F32, tag="dn")
                                for w0 in range(0, nkb, 4):
                                    wave = range(w0, min(w0 + 4, nkb))
                                    pts = {}
                                    for kb in wave:
                                        ps_sc = bps.tile([128, SC], F32, tag="sc")
                                        o = kb - c * NKBC
                                        if o >= 0:
                                            nc.tensor.matmul(
                                                ps_sc[:], ident_sb[:],
                                                masks[:, o, :],
                                                start=True, stop=False,
                                            )
                                        nc.tensor.matmul(
                                            ps_sc[:],
                                            k_sb[:, h, kb * 128:(kb + 1) * 128],
                                            q_sb[:, h, csl],
                                            start=(o < 0), stop=True,
                                        )
                                        pT = bpt.tile([128, SC], F16, tag="pt")
                                        nc.scalar.activation(
                                            pT[:], ps_sc[:], AF.Exp, scale=ISQ
                                        )
                                        pts[kb] = pT
                                    for kb in wave:
                                        nc.tensor.matmul(
                                            ps_av[:],
                                            v_sb[:, kb, h * DH:(h + 1) * DH],
                                            pts[kb][:],
                                            start=(kb == 0), stop=(kb == nkb - 1),
                                        )
                                        nc.tensor.matmul(
                                            ps_den[:], ones_sb[:], pts[kb][:],
                                            start=(kb == 0), stop=(kb == nkb - 1),
                                        )
                                den_r = bwk.tile([1, SC], F32, tag="denr")
                                nc.vector.reciprocal(den_r[:], ps_den[:])
                                dsl = slice(h * SC, (h + 1) * SC)
                                nc.sync.dma_start(den_dram[0:1, dsl], den_r[:])
                                denb = bwk.tile([128, SC], F32, tag="denb")
                                nc.sync.dma_start(
                                    denb[:], _bcast_ap(den_dram[0:1, dsl], 128)
                                )
                                nc.vector.tensor_mul(avf[:, h, :], ps_av[:], denb[:])
                            nc.sync.dma_start(ag_in[c][:], avf[:])
                            nc.gpsimd.collective_compute(
                                "AllGather",
                                mybir.AluOpType.bypass,
                                replica_groups=CGRP,
                                ins=[ag_in[c][:].opt()],
                                outs=[ag_all[c].opt()],
                            )

                    # ====== phase C: own-chunk out_proj + residual + stats ======
                    with (
                        tc.tile_pool(name="c_big", bufs=1) as cbp,
                        tc.tile_pool(name="c_work", bufs=2) as cwk,
                        tc.tile_pool(name="c_ps", bufs=2, space="PSUM") as cps,
                        tc.tile_pool(name="c_ps1", bufs=1, space="PSUM") as cps1,
                    ):
                        xr_sb = cbp.tile([128, ND, SC], F16)
                        nc.sync.dma_start(xr_sb[:], xr_in[:])
                        av_g = cbp.tile([128, ND, SC], F16)
                        tsel = nc.sync.partition_id() % G
                        src = bass.AP(
                            tensor=ag_all.tensor,
                            offset=ag_all.offset + tsel * (G * 128 * HC * SC),
                            ap=[[HC * SC, 128], [128 * HC * SC, G],
                                [SC, HC], [1, SC]],
                        )
                        nc.sync.dma_start(av_g[:], src)

                        ps_ss2 = cps1.tile([1, SC], F32, tag="ss2")
                        for m in range(ND):
                            ps_op = cps.tile([128, SC], F32, tag="op")
                            for fb in range(ND):
                                nc.tensor.matmul(
                                    ps_op[:],
                                    wout_sb[:, fb, m * 128:(m + 1) * 128],
                                    av_g[:, fb, :],
                                    start=(fb == 0), stop=(fb == ND - 1),
                                )
                            nc.vector.tensor_add(
                                out1[:, m, :], ps_op[:], xr_sb[:, m, :]
                            )
                            xsq2 = cwk.tile([128, SC], F16, tag="xsq2")
                            nc.scalar.square(xsq2[:], out1[:, m, :])
                            nc.tensor.matmul(
                                ps_ss2[:], ones_sb[:], xsq2[:],
                                start=(m == 0), stop=(m == ND - 1),
                            )
                        r2_sb = cwk.tile([1, SC], F32, tag="r2")
                        nc.scalar.activation(
                            r2_sb[:], ps_ss2[:], AF.Sqrt,
                            bias=eps_sb[:], scale=1.0 / D,
                        )
                        nc.vector.reciprocal(r2_sb[:], r2_sb[:])
                        nc.sync.dma_start(r2_dram[:], r2_sb[:])
                        nc.sync.dma_start(r2b[:], _bcast_ap(r2_dram[:], 128))

                # ============ phase D: RMSNorm2 scale + MLP ==============
                with (
                    tc.tile_pool(name="d_big", bufs=1) as dbp,
                    tc.tile_pool(name="d_stream", bufs=3) as dstr,
                    tc.tile_pool(name="d_work", bufs=2) as dwk,
                    tc.tile_pool(name="d_ps", bufs=2, space="PSUM") as dps,
                ):
                    h2 = dbp.tile([128, ND, SC], F16)
                    for d in range(ND):
                        nc.vector.tensor_mul(h2[:, d, :], out1[:, d, :], r2b[:])

                    gT = dbp.tile([128, NFF, SC], F16)
                    for m in range(NFF):
                        wm = dstr.tile([128, ND, 128], F16, tag="wfc")
                        nc.sync.dma_start(wm[:], wfc_in[m])
                        ps_fc = dps.tile([128, SC], F32, tag="fc")
                        for d in range(ND):
                            nc.tensor.matmul(
                                ps_fc[:], wm[:, d, :], h2[:, d, :],
                                start=(d == 0), stop=(d == ND - 1),
                            )
                        nc.scalar.activation(gT[:, m, :], ps_fc[:], gelu)

                    for m in range(ND):
                        wm2 = dstr.tile([128, NFF, 128], F16, tag="wproj", bufs=2)
                        nc.sync.dma_start(wm2[:], wproj_in[m])
                        ps_pr = dps.tile([128, SC], F32, tag="pr")
                        for k in range(NFF):
                            nc.tensor.matmul(
                                ps_pr[:], wm2[:, k, :], gT[:, k, :],
                                start=(k == 0), stop=(k == NFF - 1),
                            )
                        o2 = dwk.tile([128, SC], F32, tag="o2")
                        nc.vector.tensor_add(o2[:], ps_pr[:], out1[:, m, :])
                        nc.sync.dma_start(out_ext[m], o2[:])

    nc.compile()
    return nc


def _deinterleave(w):
    """Reorder head-dim columns: evens then odds (per 128-wide head)."""
    Din, Dout = w.shape
    nh = Dout // DH
    w4 = w.reshape(Din, nh, DH // 2, 2)
    return np.concatenate([w4[..., 0], w4[..., 1]], axis=2).reshape(Din, Dout)


def prep_inputs(x, w_qkv, w_out, w_fc, w_proj, g_in, g_ff, S, D, H, FF):
    HC = H // G
    SC = S // 4
    ND = D // 128
    NQK = 2 * HC
    NV = HC * DH
    NKBC = SC // 128
    NFF = FF // 128

    x = np.asarray(x, np.float32)
    w_qkv = np.asarray(w_qkv, np.float32)
    w_out = np.asarray(w_out, np.float32)
    w_fc = np.asarray(w_fc, np.float32)
    w_proj = np.asarray(w_proj, np.float32)
    g_in = np.asarray(g_in, np.float32)
    g_ff = np.asarray(g_ff, np.float32)

    wq = w_qkv * g_in[:, None]

    half = DH // 2
    invf = 1.0 / (ROPE_BASE ** (2.0 * np.arange(half, dtype=np.float64) / DH))
    ang = np.arange(S, dtype=np.float64)[:, None] * invf[None, :]
    ctab = np.ascontiguousarray(np.cos(ang).T.astype(np.float32))
    stab = np.ascontiguousarray(np.sin(ang).T.astype(np.float32))

    masks = np.zeros((NKBC, 128, SC), np.float32)
    pp = np.arange(128)[:, None]
    qq = np.arange(SC)[None, :]
    for o in range(NKBC):
        masks[o] = np.where(o * 128 + pp <= qq, 0.0, -60000.0).astype(np.float32)
    masks = np.ascontiguousarray(masks.transpose(1, 0, 2)).astype(np.float16)
    ident = np.eye(128, dtype=np.float16)

    wfc = w_fc * g_ff[:, None]
    wfc_r = np.ascontiguousarray(
        wfc.reshape(ND, 128, NFF, 128).transpose(2, 1, 0, 3)
    ).astype(np.float16)
    wproj_r = np.ascontiguousarray(
        w_proj.reshape(NFF, 128, ND, 128).transpose(2, 1, 0, 3)
    ).astype(np.float16)
    wout_r = np.ascontiguousarray(
        w_out.reshape(ND, 128, D).transpose(1, 0, 2)
    ).astype(np.float16)
    ones = np.ones((128, 1), np.float16)

    in_maps = []
    for core in range(NCORES):
        b, t = core // G, core % G
        xb = x[b]
        xT = np.ascontiguousarray(xb.T)
        xT_r = np.ascontiguousarray(
            xT.reshape(ND, 128, 4, SC).transpose(2, 1, 0, 3)
        ).astype(np.float16)
        xr = np.ascontiguousarray(
            xb[t * SC:(t + 1) * SC, :].T.reshape(ND, 128, SC).transpose(1, 0, 2)
        ).astype(np.float16)
        qcols = _deinterleave(wq[:, t * NV:(t + 1) * NV])
        kcols = _deinterleave(wq[:, D + t * NV:D + (t + 1) * NV])
        vcols = wq[:, 2 * D + t * NV:2 * D + (t + 1) * NV]
        wqk_core = np.ascontiguousarray(
            np.concatenate([qcols, kcols], axis=1)
            .reshape(ND, 128, NQK, 128).transpose(2, 1, 0, 3)
        ).astype(np.float16)
        wv_core = np.ascontiguousarray(
            vcols.reshape(ND, 128, NV).transpose(1, 0, 2)
        ).astype(np.float16)
        in_maps.append({
            "xT": xT_r, "xr": xr, "wqk": wqk_core, "wv": wv_core,
            "wout": wout_r, "wfc": wfc_r, "wproj": wproj_r,
            "ctab": ctab, "stab": stab, "masks": masks, "ones": ones,
            "ident": ident,
        })
    return in_maps


def assemble(results, S, D):
    SC = S // 4
    y = np.zeros((2, S, D), np.float32)
    for core in range(NCORES):
        b, t = core // G, core % G
        o = results[core]["out"]
        y[b, t * SC:(t + 1) * SC, :] = o.reshape(D, SC).T
    return y


_CACHE = {}


def run(inputs, S, D, H, FF, trace=False, **kw):
    key = (S, D, H, FF)
    if key not in _CACHE:
        _CACHE[key] = build_nc(S, D, H, FF)
    nc = _CACHE[key]
    in_maps = prep_inputs(
        inputs["x"], inputs["w_qkv"], inputs["w_out"], inputs["w_fc"],
        inputs["w_proj"], inputs["g_in"], inputs["g_ff"], S, D, H, FF,
    )
    res = run_bass_kernel_spmd(nc, in_maps, list(range(NCORES)), trace=trace, **kw)
    return assemble(res.results, S, D), res


def kernel(**inputs):
    y, _ = run(inputs, S=2048, D=2048, H=16, FF=4096)
    return y.astype(np.float32)


# revision 13
# speedup vs baseline: 1.2383x; 1.0612x over previous
"""Trainium2 Bass kernel for one dense transformer block (RMSNorm -> causal
RoPE attention -> residual -> RMSNorm -> GELU MLP -> residual).

Sharding across 8 NeuronCores: 2 batch-groups (data parallel over B=2) x 4
ranks. Within a group: tensor-parallel over heads for QKV+attention. The
normalized per-head attention outputs (fp16, S x D/4 per rank) are exchanged
with four small per-chunk AllGathers issued inside the attention phase (so
they overlap compute); each rank then reads its own 512-token chunk of the
gathered buffer via a partition_id-driven dynamic DMA offset and runs
out_proj + RMSNorm + MLP + residuals for that chunk with full weights.

Layout/precision notes:
- All matmul operands are fp16 (f32 PSUM accumulation); fp16 LDWEIGHTS is
  ~4x faster than f32r, which lifts the back-to-back matmul issue rate.
- Q/K weight columns are de-interleaved per head on the host (even rotary
  dims -> partitions 0..63, odd -> 64..127) so RoPE is contiguous DVE math.
- RMSNorm scales fold into the RoPE tables (q,k) / a per-token PSUM-copy
  scale (v); g_in/g_ff fold into the weights on the host.
- Q/K/V live in SBUF between phases (no DRAM round-trip).
- Softmax skips the max-subtraction (scores are ~N(0,1); exp cannot
  overflow) so denominators come from ones-vector matmuls on the PE.
"""

import os
import sys

import numpy as np

for _p in ("/root/.axon_site/_ro/trn_rl_repo", "/opt/trn_rl_repo"):
    if os.path.isdir(_p) and _p not in sys.path:
        sys.path.append(_p)

import concourse.bass as bass  # noqa: E402
import concourse.mybir as mybir  # noqa: E402
import concourse.tile as tile  # noqa: E402
from concourse import bacc  # noqa: E402
from concourse.bass_utils import run_bass_kernel_spmd  # noqa: E402

F32 = mybir.dt.float32
F16 = mybir.dt.float16
AF = mybir.ActivationFunctionType

G = 4  # ranks per batch-group
NCORES = 8
DH = 128  # head dim (= partition width)
EPS = 1e-6
ROPE_BASE = 10000.0
CGRP = [[0, 1, 2, 3], [4, 5, 6, 7]]


def _bcast_ap(ap, parts):
    """Partition-broadcast view of a [1, N] AP -> [parts, N]."""
    a2 = ap.opt()
    return bass.AP(tensor=a2.tensor, offset=a2.offset, ap=[[0, parts]] + list(a2.ap))


def build_nc(S, D, H, FF, gelu=None):
    gelu = gelu if gelu is not None else AF.Gelu_apprx_tanh
    HC = H // G  # heads per core
    SC = S // 4  # chunk length == sequence shard length
    ND = D // 128
    NQK = 2 * HC  # q then k feature blocks (128 wide)
    NV = HC * DH
    NKBC = SC // 128  # 128-token k-blocks per chunk
    NB = S // 128
    NFF = FF // 128
    ISQ = float(1.0 / np.sqrt(DH))

    nc = bacc.Bacc("TRN2", target_bir_lowering=False, debug=False, num_devices=NCORES)

    xT_in = nc.dram_tensor("xT", [4, 128, ND, SC], F16, kind="ExternalInput")
    xr_in = nc.dram_tensor("xr", [128, ND, SC], F16, kind="ExternalInput")
    wqk_in = nc.dram_tensor("wqk", [NQK, 128, ND, 128], F16, kind="ExternalInput")
    wv_in = nc.dram_tensor("wv", [128, ND, NV], F16, kind="ExternalInput")
    wout_in = nc.dram_tensor("wout", [128, ND, D], F16, kind="ExternalInput")
    wfc_in = nc.dram_tensor("wfc", [NFF, 128, ND, 128], F16, kind="ExternalInput")
    wproj_in = nc.dram_tensor("wproj", [ND, 128, NFF, 128], F16, kind="ExternalInput")
    ctab_in = nc.dram_tensor("ctab", [64, S], F32, kind="ExternalInput")
    stab_in = nc.dram_tensor("stab", [64, S], F32, kind="ExternalInput")
    mask_in = nc.dram_tensor("masks", [128, NKBC, SC], F16, kind="ExternalInput")
    ident_in = nc.dram_tensor("ident", [128, 128], F16, kind="ExternalInput")
    ones_in = nc.dram_tensor("ones", [128, 1], F16, kind="ExternalInput")
    onesr_in = nc.dram_tensor("onesr", [1, 128], F16, kind="ExternalInput")
    out_ext = nc.dram_tensor("out", [ND, 128, SC], F32, kind="ExternalOutput")

    with tile.TileContext(nc) as tc:
        with (
            tc.tile_pool(name="const", bufs=1) as constp,
            tc.tile_pool(name="dram", bufs=1, space="DRAM") as dramp,
            tc.tile_pool(name="kvq", bufs=1) as kvqp,
        ):
            r_dram = dramp.tile([1, S], F32)
            den_dram = dramp.tile([1, HC * SC], F32)
            r2_dram = dramp.tile([1, SC], F32)
            ag_in = [dramp.tile([128, HC, SC], F16, name=f"agi{c}", tag=f"agi{c}")
                     for c in range(4)]
            ag_all = dramp.tile([4, G, 128, HC, SC], F16)

            ones_sb = constp.tile([128, 1], F16)
            nc.sync.dma_start(ones_sb[:], ones_in[:])
            ident_sb = constp.tile([128, 128], F16)
            nc.sync.dma_start(ident_sb[:], ident_in[:])
            onesr_sb = constp.tile([1, 128], F16)
            nc.sync.dma_start(onesr_sb[:], onesr_in[:])
            eps_sb = constp.tile([1, 1], F32)
            nc.vector.memset(eps_sb[:], EPS)

            k_sb = kvqp.tile([128, HC, S], F16)
            q_sb = kvqp.tile([128, HC, S], F16)
            v_sb = kvqp.tile([128, NB, NV], F16)

            # ================= phase A: stats + QKV + RoPE =================
            with (
                tc.tile_pool(name="a_w", bufs=1) as awp,
                tc.tile_pool(name="a_stream", bufs=3) as astr,
                tc.tile_pool(name="a_x", bufs=2) as axp,
                tc.tile_pool(name="a_work", bufs=2) as awk,
                tc.tile_pool(name="a_ps", bufs=4, space="PSUM") as apsum,
                tc.tile_pool(name="a_ps1", bufs=1, space="PSUM") as apsum1,
            ):
                wv_sb = awp.tile([128, ND, NV], F16)
                ctab_sb = awp.tile([64, S], F32)
                stab_sb = awp.tile([64, S], F32)

                for c in range(4):
                    csl = slice(c * SC, (c + 1) * SC)
                    x_sb = axp.tile([128, ND, SC], F16, tag="xchunk")
                    for dd in range(0, ND, 4):
                        nc.sync.dma_start(
                            x_sb[:, dd:dd + 4, :], xT_in[c][:, dd:dd + 4, :]
                        )

                    # squares stream on the scalar engine while x arrives
                    xsq = awk.tile([128, ND, SC], F16, tag="xsq", bufs=1)
                    for d in range(ND):
                        nc.scalar.square(xsq[:, d, :], x_sb[:, d, :])

                    # first q block's matmuls keep the PE busy while the
                    # squares drain; stats matmuls follow them
                    wm0 = astr.tile([128, ND, 128], F16, tag="wqk")
                    nc.sync.dma_start(wm0[:], wqk_in[0])
                    if c == 0:
                        nc.sync.dma_start(ctab_sb[:], ctab_in[:])
                        nc.sync.dma_start(stab_sb[:], stab_in[:])
                    ps_qk0 = apsum.tile([128, SC], F32, tag="qk")
                    for d in range(ND):
                        nc.tensor.matmul(
                            ps_qk0[:], wm0[:, d, :], x_sb[:, d, :],
                            start=(d == 0), stop=(d == ND - 1),
                        )
                    ps_ss = apsum1.tile([1, SC], F32, tag="ss")
                    for d in range(ND):
                        nc.tensor.matmul(
                            ps_ss[:], ones_sb[:], xsq[:, d, :],
                            start=(d == 0), stop=(d == ND - 1),
                        )
                    r_sb = awk.tile([1, SC], F32, tag="rsb")
                    nc.scalar.activation(
                        r_sb[:], ps_ss[:], AF.Sqrt, bias=eps_sb[:], scale=1.0 / D
                    )
                    nc.vector.reciprocal(r_sb[:], r_sb[:])
                    nc.sync.dma_start(r_dram[0:1, csl], r_sb[:])

                    r64 = awk.tile([64, SC], F32, tag="r64")
                    nc.sync.dma_start(r64[:], _bcast_ap(r_dram[0:1, csl], 64))
                    r_tok = awk.tile([128, NKBC], F32, tag="rtok")
                    flat = r_dram[0:1, csl]
                    nc.sync.dma_start(
                        r_tok[:],
                        bass.AP(tensor=flat.tensor, offset=flat.offset,
                                ap=[[1, 128], [128, NKBC]]),
                    )

                    ctr = awk.tile([64, SC], F32, tag="ctr")
                    srt = awk.tile([64, SC], F32, tag="srt")
                    nc.vector.tensor_mul(ctr[:], ctab_sb[:, csl], r64[:])
                    nc.vector.tensor_mul(srt[:], stab_sb[:, csl], r64[:])

                    # q/k matmuls + rope (features de-interleaved on host:
                    # rotary-even dims on partitions 0:64, odd on 64:128)
                    def rope(ps_qk, m):
                        dst = q_sb if m < HC else k_sb
                        hh = m if m < HC else m - HC
                        t1 = awk.tile([64, SC], F32, tag="t1")
                        t2 = awk.tile([64, SC], F32, tag="t2")
                        nc.vector.tensor_mul(t1[:], ps_qk[0:64, :], ctr[:])
                        nc.vector.tensor_mul(t2[:], ps_qk[64:128, :], srt[:])
                        nc.vector.tensor_sub(dst[0:64, hh, csl], t1[:], t2[:])
                        nc.vector.tensor_mul(t1[:], ps_qk[64:128, :], ctr[:])
                        nc.vector.tensor_mul(t2[:], ps_qk[0:64, :], srt[:])
                        nc.vector.tensor_add(dst[64:128, hh, csl], t1[:], t2[:])

                    rope(ps_qk0, 0)
                    for m in range(1, NQK):
                        wm = astr.tile([128, ND, 128], F16, tag="wqk")
                        nc.sync.dma_start(wm[:], wqk_in[m])
                        if c == 0 and m == 1:
                            nc.sync.dma_start(wv_sb[:], wv_in[:])
                        ps_qk = apsum.tile([128, SC], F32, tag="qk")
                        for d in range(ND):
                            nc.tensor.matmul(
                                ps_qk[:], wm[:, d, :], x_sb[:, d, :],
                                start=(d == 0), stop=(d == ND - 1),
                            )
                        rope(ps_qk, m)

                    # v matmuls (token-major), fold rms scale into PSUM copy
                    for sb_i in range(NKBC):
                        ps_v = apsum.tile([128, NV], F32, tag="v", bufs=3)
                        for d in range(ND):
                            nc.tensor.matmul(
                                ps_v[:], x_sb[:, d, sb_i * 128:(sb_i + 1) * 128],
                                wv_sb[:, d, :],
                                start=(d == 0), stop=(d == ND - 1),
                            )
                        nc.scalar.activation(
                            v_sb[:, c * NKBC + sb_i, :], ps_v[:], AF.Copy,
                            scale=r_tok[:, sb_i:sb_i + 1],
                        )

            # ====== residual tile lives from phase C into phase D ======
            with tc.tile_pool(name="res", bufs=1) as resp:
                out1 = resp.tile([128, ND, SC], F32)
                r2h = resp.tile([1, SC], F16)

                with tc.tile_pool(name="wout", bufs=1) as woutp:
                    wout_sb = woutp.tile([128, ND, D], F16)

                    # ========== phase B: attention + per-chunk AllGather ==========
                    with (
                        tc.tile_pool(name="b_m", bufs=1) as bmp,
                        tc.tile_pool(name="b_work", bufs=2) as bwk,
                        tc.tile_pool(name="b_pt", bufs=8) as bpt,
                        tc.tile_pool(name="b_av", bufs=2) as bav,
                        tc.tile_pool(name="b_ps", bufs=4, space="PSUM") as bps,
                        tc.tile_pool(name="b_ps_acc", bufs=2, space="PSUM") as bpsa,
                    ):
                        masks = bmp.tile([128, NKBC, SC], F16)
                        nc.sync.dma_start(masks[:], mask_in[:])
                        nc.sync.dma_start(wout_sb[:], wout_in[:])

                        for c in range(4):
                            csl = slice(c * SC, (c + 1) * SC)
                            nkb = (c + 1) * NKBC
                            avf = bav.tile([128, HC, SC], F16, tag="avf")
                            for h in range(HC):
                                ps_av = bpsa.tile([128, SC], F32, tag="av")
                                ps_den = bpsa.tile([1, SC], F32, tag="dn")
                                for w0 in range(0, nkb, 4):
                                    wave = range(w0, min(w0 + 4, nkb))
                                    pts = {}
                                    for kb in wave:
                                        ps_sc = bps.tile([128, SC], F32, tag="sc")
                                        o = kb - c * NKBC
                                        if o >= 0:
                                            nc.tensor.matmul(
                                                ps_sc[:], ident_sb[:],
                                                masks[:, o, :],
                                                start=True, stop=False,
                                            )
                                        nc.tensor.matmul(
                                            ps_sc[:],
                                            k_sb[:, h, kb * 128:(kb + 1) * 128],
                                            q_sb[:, h, csl],
                                            start=(o < 0), stop=True,
                                        )
                                        pT = bpt.tile([128, SC], F16, tag="pt")
                                        nc.scalar.activation(
                                            pT[:], ps_sc[:], AF.Exp, scale=ISQ
                                        )
                                        pts[kb] = pT
                                    for kb in wave:
                                        nc.tensor.matmul(
                                            ps_av[:],
                                            v_sb[:, kb, h * DH:(h + 1) * DH],
                                            pts[kb][:],
                                            start=(kb == 0), stop=(kb == nkb - 1),
                                        )
                                        nc.tensor.matmul(
                                            ps_den[:], ones_sb[:], pts[kb][:],
                                            start=(kb == 0), stop=(kb == nkb - 1),
                                        )
                                den_r = bwk.tile([1, SC], F32, tag="denr")
                                nc.vector.reciprocal(den_r[:], ps_den[:])
                                dsl = slice(h * SC, (h + 1) * SC)
                                nc.sync.dma_start(den_dram[0:1, dsl], den_r[:])
                                denb = bwk.tile([128, SC], F32, tag="denb")
                                nc.sync.dma_start(
                                    denb[:], _bcast_ap(den_dram[0:1, dsl], 128)
                                )
                                nc.vector.tensor_mul(avf[:, h, :], ps_av[:], denb[:])
                                nc.sync.dma_start(ag_in[c][:, h, :], avf[:, h, :])
                            nc.gpsimd.collective_compute(
                                "AllGather",
                                mybir.AluOpType.bypass,
                                replica_groups=CGRP,
                                ins=[ag_in[c][:].opt()],
                                outs=[ag_all[c].opt()],
                            )

                    # ====== phase C: own-chunk out_proj + residual + stats ======
                    with (
                        tc.tile_pool(name="c_big", bufs=1) as cbp,
                        tc.tile_pool(name="c_work", bufs=2) as cwk,
                        tc.tile_pool(name="c_ps", bufs=2, space="PSUM") as cps,
                        tc.tile_pool(name="c_ps1", bufs=1, space="PSUM") as cps1,
                    ):
                        xr_sb = cbp.tile([128, ND, SC], F16)
                        nc.sync.dma_start(xr_sb[:], xr_in[:])
                        av_g = cbp.tile([128, ND, SC], F16)
                        tsel = nc.sync.partition_id() % G
                        src = bass.AP(
                            tensor=ag_all.tensor,
                            offset=ag_all.offset + tsel * (G * 128 * HC * SC),
                            ap=[[HC * SC, 128], [128 * HC * SC, G],
                                [SC, HC], [1, SC]],
                        )
                        nc.sync.dma_start(av_g[:], src)

                        ps_ss2 = cps1.tile([1, SC], F32, tag="ss2")
                        xsqs = []
                        for m in range(ND):
                            ps_op = cps.tile([128, SC], F32, tag="op")
                            for fb in range(ND):
                                nc.tensor.matmul(
                                    ps_op[:],
                                    wout_sb[:, fb, m * 128:(m + 1) * 128],
                                    av_g[:, fb, :],
                                    start=(fb == 0), stop=(fb == ND - 1),
                                )
                            # stats matmul for the previous block: keeps the
                            # scalar-engine square off the PE critical path
                            if m > 0:
                                nc.tensor.matmul(
                                    ps_ss2[:], ones_sb[:], xsqs[m - 1][:],
                                    start=(m == 1), stop=False,
                                )
                            nc.vector.tensor_add(
                                out1[:, m, :], ps_op[:], xr_sb[:, m, :]
                            )
                            xsq2 = cwk.tile([128, SC], F16, tag="xsq2")
                            nc.scalar.square(xsq2[:], out1[:, m, :])
                            xsqs.append(xsq2)
                        nc.tensor.matmul(
                            ps_ss2[:], ones_sb[:], xsqs[ND - 1][:],
                            start=False, stop=True,
                        )
                        r2_sb = cwk.tile([1, SC], F32, tag="r2")
                        nc.scalar.activation(
                            r2_sb[:], ps_ss2[:], AF.Sqrt,
                            bias=eps_sb[:], scale=1.0 / D,
                        )
                        nc.vector.reciprocal(r2_sb[:], r2_sb[:])
                        nc.scalar.copy(r2h[:], r2_sb[:])

                # ============ phase D: RMSNorm2 scale + MLP ==============
                with (
                    tc.tile_pool(name="d_big", bufs=1) as dbp,
                    tc.tile_pool(name="d_stream", bufs=3) as dstr,
                    tc.tile_pool(name="d_work", bufs=2) as dwk,
                    tc.tile_pool(name="d_ps", bufs=3, space="PSUM") as dps,
                ):
                    # broadcast 1/rms2 across partitions via a ones outer
                    # product on the PE (faster than a DRAM round-trip)
                    ps_r2b = dps.tile([128, SC], F32, tag="r2b", bufs=1)
                    nc.tensor.matmul(
                        ps_r2b[:], onesr_sb[:], r2h[:], start=True, stop=True
                    )
                    h2 = dbp.tile([128, ND, SC], F16)
                    for d in range(ND):
                        nc.vector.tensor_mul(h2[:, d, :], out1[:, d, :], ps_r2b[:])

                    gT = dbp.tile([128, NFF, SC], F16)
                    for m in range(NFF):
                        wm = dstr.tile([128, ND, 128], F16, tag="wfc")
                        nc.sync.dma_start(wm[:], wfc_in[m])
                        ps_fc = dps.tile([128, SC], F32, tag="fc")
                        for d in range(ND):
                            nc.tensor.matmul(
                                ps_fc[:], wm[:, d, :], h2[:, d, :],
                                start=(d == 0), stop=(d == ND - 1),
                            )
                        nc.scalar.activation(gT[:, m, :], ps_fc[:], gelu)

                    for m in range(ND):
                        wm2 = dstr.tile([128, NFF, 128], F16, tag="wproj", bufs=2)
                        nc.sync.dma_start(wm2[:], wproj_in[m])
                        ps_pr = dps.tile([128, SC], F32, tag="pr")
                        for k in range(NFF):
                            nc.tensor.matmul(
                                ps_pr[:], wm2[:, k, :], gT[:, k, :],
                                start=(k == 0), stop=(k == NFF - 1),
                            )
                        o2 = dwk.tile([128, SC], F32, tag="o2")
                        nc.vector.tensor_add(o2[:], ps_pr[:], out1[:, m, :])
                        nc.sync.dma_start(out_ext[m], o2[:])

    nc.compile()
    return nc


def _deinterleave(w):
    """Reorder head-dim columns: evens then odds (per 128-wide head)."""
    Din, Dout = w.shape
    nh = Dout // DH
    w4 = w.reshape(Din, nh, DH // 2, 2)
    return np.concatenate([w4[..., 0], w4[..., 1]], axis=2).reshape(Din, Dout)


def prep_inputs(x, w_qkv, w_out, w_fc, w_proj, g_in, g_ff, S, D, H, FF):
    HC = H // G
    SC = S // 4
    ND = D // 128
    NQK = 2 * HC
    NV = HC * DH
    NKBC = SC // 128
    NFF = FF // 128

    x = np.asarray(x, np.float32)
    w_qkv = np.asarray(w_qkv, np.float32)
    w_out = np.asarray(w_out, np.float32)
    w_fc = np.asarray(w_fc, np.float32)
    w_proj = np.asarray(w_proj, np.float32)
    g_in = np.asarray(g_in, np.float32)
    g_ff = np.asarray(g_ff, np.float32)

    wq = w_qkv * g_in[:, None]

    half = DH // 2
    invf = 1.0 / (ROPE_BASE ** (2.0 * np.arange(half, dtype=np.float64) / DH))
    ang = np.arange(S, dtype=np.float64)[:, None] * invf[None, :]
    ctab = np.ascontiguousarray(np.cos(ang).T.astype(np.float32))
    stab = np.ascontiguousarray(np.sin(ang).T.astype(np.float32))

    masks = np.zeros((NKBC, 128, SC), np.float32)
    pp = np.arange(128)[:, None]
    qq = np.arange(SC)[None, :]
    for o in range(NKBC):
        masks[o] = np.where(o * 128 + pp <= qq, 0.0, -60000.0).astype(np.float32)
    masks = np.ascontiguousarray(masks.transpose(1, 0, 2)).astype(np.float16)
    ident = np.eye(128, dtype=np.float16)

    wfc = w_fc * g_ff[:, None]
    wfc_r = np.ascontiguousarray(
        wfc.reshape(ND, 128, NFF, 128).transpose(2, 1, 0, 3)
    ).astype(np.float16)
    wproj_r = np.ascontiguousarray(
        w_proj.reshape(NFF, 128, ND, 128).transpose(2, 1, 0, 3)
    ).astype(np.float16)
    wout_r = np.ascontiguousarray(
        w_out.reshape(ND, 128, D).transpose(1, 0, 2)
    ).astype(np.float16)
    ones = np.ones((128, 1), np.float16)
    onesr = np.ones((1, 128), np.float16)

    in_maps = []
    for core in range(NCORES):
        b, t = core // G, core % G
        xb = x[b]
        xT = np.ascontiguousarray(xb.T)
        xT_r = np.ascontiguousarray(
            xT.reshape(ND, 128, 4, SC).transpose(2, 1, 0, 3)
        ).astype(np.float16)
        xr = np.ascontiguousarray(
            xb[t * SC:(t + 1) * SC, :].T.reshape(ND, 128, SC).transpose(1, 0, 2)
        ).astype(np.float16)
        qcols = _deinterleave(wq[:, t * NV:(t + 1) * NV])
        kcols = _deinterleave(wq[:, D + t * NV:D + (t + 1) * NV])
        vcols = wq[:, 2 * D + t * NV:2 * D + (t + 1) * NV]
        wqk_core = np.ascontiguousarray(
            np.concatenate([qcols, kcols], axis=1)
            .reshape(ND, 128, NQK, 128).transpose(2, 1, 0, 3)
        ).astype(np.float16)
        wv_core = np.ascontiguousarray(
            vcols.reshape(ND, 128, NV).transpose(1, 0, 2)
        ).astype(np.float16)
        in_maps.append({
            "xT": xT_r, "xr": xr, "wqk": wqk_core, "wv": wv_core,
            "wout": wout_r, "wfc": wfc_r, "wproj": wproj_r,
            "ctab": ctab, "stab": stab, "masks": masks, "ones": ones,
            "onesr": onesr, "ident": ident,
        })
    return in_maps


def assemble(results, S, D):
    SC = S // 4
    y = np.zeros((2, S, D), np.float32)
    for core in range(NCORES):
        b, t = core // G, core % G
        o = results[core]["out"]
        y[b, t * SC:(t + 1) * SC, :] = o.reshape(D, SC).T
    return y


_CACHE = {}


def run(inputs, S, D, H, FF, trace=False, **kw):
    key = (S, D, H, FF)
    if key not in _CACHE:
        _CACHE[key] = build_nc(S, D, H, FF)
    nc = _CACHE[key]
    in_maps = prep_inputs(
        inputs["x"], inputs["w_qkv"], inputs["w_out"], inputs["w_fc"],
        inputs["w_proj"], inputs["g_in"], inputs["g_ff"], S, D, H, FF,
    )
    res = run_bass_kernel_spmd(nc, in_maps, list(range(NCORES)), trace=trace, **kw)
    return assemble(res.results, S, D), res


def kernel(**inputs):
    y, _ = run(inputs, S=2048, D=2048, H=16, FF=4096)
    return y.astype(np.float32)


# revision 29
# speedup vs baseline: 1.2689x; 1.0247x over previous
"""Trainium2 Bass kernel for one dense transformer block (RMSNorm -> causal
RoPE attention -> residual -> RMSNorm -> GELU MLP -> residual).

Sharding across 8 NeuronCores: 2 batch-groups (data parallel over B=2) x 4
ranks. Within a group: tensor-parallel over heads for QKV+attention. The
normalized per-head attention outputs (fp16, S x D/4 per rank) are exchanged
with four small per-chunk AllGathers issued inside the attention phase (so
they overlap compute); each rank then reads its own 512-token chunk of the
gathered buffer via a partition_id-driven dynamic DMA offset and runs
out_proj + RMSNorm + MLP + residuals for that chunk with full weights.

Layout/precision notes:
- All matmul operands are fp16 (f32 PSUM accumulation); fp16 LDWEIGHTS is
  ~4x faster than f32r, which lifts the back-to-back matmul issue rate.
- Q/K weight columns are de-interleaved per head on the host (even rotary
  dims -> partitions 0..63, odd -> 64..127) so RoPE is contiguous DVE math.
- RMSNorm scales fold into the RoPE tables (q,k) / a per-token PSUM-copy
  scale (v); g_in/g_ff fold into the weights on the host.
- Q/K/V live in SBUF between phases (no DRAM round-trip).
- Softmax skips the max-subtraction (scores are ~N(0,1); exp cannot
  overflow) so denominators come from ones-vector matmuls on the PE.
"""

import os
import sys

import numpy as np

for _p in ("/root/.axon_site/_ro/trn_rl_repo", "/opt/trn_rl_repo"):
    if os.path.isdir(_p) and _p not in sys.path:
        sys.path.append(_p)

import concourse.bass as bass  # noqa: E402
import concourse.mybir as mybir  # noqa: E402
import concourse.tile as tile  # noqa: E402
from concourse import bacc  # noqa: E402
from concourse.bass_utils import run_bass_kernel_spmd  # noqa: E402

F32 = mybir.dt.float32
F16 = mybir.dt.float16
AF = mybir.ActivationFunctionType

G = 4  # ranks per batch-group
NCORES = 8
DH = 128  # head dim (= partition width)
EPS = 1e-6
ROPE_BASE = 10000.0
CGRP = [[0, 1, 2, 3], [4, 5, 6, 7]]


def _bcast_ap(ap, parts):
    """Partition-broadcast view of a [1, N] AP -> [parts, N]."""
    a2 = ap.opt()
    return bass.AP(tensor=a2.tensor, offset=a2.offset, ap=[[0, parts]] + list(a2.ap))


def build_nc(S, D, H, FF, gelu=None):
    gelu = gelu if gelu is not None else AF.Gelu_apprx_tanh
    HC = H // G  # heads per core
    SC = S // 4  # chunk length == sequence shard length
    ND = D // 128
    NQK = 2 * HC  # q then k feature blocks (128 wide)
    NV = HC * DH
    NKBC = SC // 128  # 128-token k-blocks per chunk
    NB = S // 128
    NFF = FF // 128
    ISQ = float(1.0 / np.sqrt(DH))
    EXPB = float(-np.log(16.0))  # softmax-invariant exp bias: keeps fp16 den small

    nc = bacc.Bacc("TRN2", target_bir_lowering=False, debug=False, num_devices=NCORES)

    xT_in = nc.dram_tensor("xT", [4, 128, ND, SC], F16, kind="ExternalInput")
    xr_in = nc.dram_tensor("xr", [128, ND, SC], F16, kind="ExternalInput")
    wqk_in = nc.dram_tensor("wqk", [NQK, 128, ND, 128], F16, kind="ExternalInput")
    wv_in = nc.dram_tensor("wv", [128, ND, NV], F16, kind="ExternalInput")
    wout_in = nc.dram_tensor("wout", [128, ND, D], F16, kind="ExternalInput")
    wfc_in = nc.dram_tensor("wfc", [NFF, 128, ND, 128], F16, kind="ExternalInput")
    wproj_in = nc.dram_tensor("wproj", [ND, 128, NFF, 128], F16, kind="ExternalInput")
    ctab_in = nc.dram_tensor("ctab", [64, S], F32, kind="ExternalInput")
    stab_in = nc.dram_tensor("stab", [64, S], F32, kind="ExternalInput")
    mask_in = nc.dram_tensor("masks", [128, NKBC, SC], F16, kind="ExternalInput")
    ident_in = nc.dram_tensor("ident", [128, 128], F16, kind="ExternalInput")
    ones_in = nc.dram_tensor("ones", [128, 1], F16, kind="ExternalInput")
    onesr_in = nc.dram_tensor("onesr", [1, 128], F16, kind="ExternalInput")
    out_ext = nc.dram_tensor("out", [ND, 128, SC], F32, kind="ExternalOutput")

    with tile.TileContext(nc) as tc:
        with (
            tc.tile_pool(name="const", bufs=1) as constp,
            tc.tile_pool(name="dram", bufs=1, space="DRAM") as dramp,
            tc.tile_pool(name="kvq", bufs=1) as kvqp,
        ):
            r_dram = dramp.tile([1, S], F32)
            SCH = SC // 2
            ag_in = [dramp.tile([128, HC, SC], F16, name=f"agi{c}", tag=f"agi{c}")
                     for c in range(4)]
            ag_all = dramp.tile([4, G, 128, HC, SC], F16)

            ones_sb = constp.tile([128, 1], F16)
            nc.sync.dma_start(ones_sb[:], ones_in[:])
            ident_sb = constp.tile([128, 128], F16)
            nc.sync.dma_start(ident_sb[:], ident_in[:])
            onesr_sb = constp.tile([1, 128], F16)
            nc.sync.dma_start(onesr_sb[:], onesr_in[:])
            eps_sb = constp.tile([1, 1], F32)
            nc.vector.memset(eps_sb[:], EPS)
            expb_sb = constp.tile([128, 1], F32)
            nc.vector.memset(expb_sb[:], EXPB)

            k_sb = kvqp.tile([128, HC, S], F16)
            q_sb = kvqp.tile([128, HC, S], F16)
            v_sb = kvqp.tile([128, NB, NV], F16)

            # ================= phase A: stats + QKV + RoPE =================
            with (
                tc.tile_pool(name="a_w", bufs=1) as awp,
                tc.tile_pool(name="a_stream", bufs=3) as astr,
                tc.tile_pool(name="a_x", bufs=2) as axp,
                tc.tile_pool(name="a_work", bufs=2) as awk,
                tc.tile_pool(name="a_ps", bufs=4, space="PSUM") as apsum,
                tc.tile_pool(name="a_ps1", bufs=1, space="PSUM") as apsum1,
            ):
                wv_sb = awp.tile([128, ND, NV], F16)
                ctab_sb = awp.tile([64, S], F32)
                stab_sb = awp.tile([64, S], F32)

                for c in range(4):
                    csl = slice(c * SC, (c + 1) * SC)
                    x_sb = axp.tile([128, ND, SC], F16, tag="xchunk")
                    for dd in range(0, ND, 4):
                        nc.sync.dma_start(
                            x_sb[:, dd:dd + 4, :], xT_in[c][:, dd:dd + 4, :]
                        )

                    # squares stream on the scalar engine while x arrives
                    xsq = awk.tile([128, ND, SC], F16, tag="xsq", bufs=1)
                    for d in range(ND):
                        nc.scalar.square(xsq[:, d, :], x_sb[:, d, :])

                    # first q block's matmuls keep the PE busy while the
                    # squares drain; stats matmuls follow them
                    wm0 = astr.tile([128, ND, 128], F16, tag="wqk")
                    nc.sync.dma_start(wm0[:], wqk_in[0])
                    if c == 0:
                        nc.sync.dma_start(ctab_sb[:], ctab_in[:])
                        nc.sync.dma_start(stab_sb[:], stab_in[:])
                    ps_qk0 = apsum.tile([128, SC], F32, tag="qk")
                    for d in range(ND):
                        nc.tensor.matmul(
                            ps_qk0[:], wm0[:, d, :], x_sb[:, d, :],
                            start=(d == 0), stop=(d == ND - 1),
                        )
                    ps_ss = apsum1.tile([1, SC], F32, tag="ss")
                    for d in range(ND):
                        nc.tensor.matmul(
                            ps_ss[:], ones_sb[:], xsq[:, d, :],
                            start=(d == 0), stop=(d == ND - 1),
                        )
                    # rms = sqrt(ms); broadcast over partitions via a ones
                    # outer product, then a partition-parallel reciprocal
                    # (a [1,512] DVE reciprocal is serial and slow)
                    r_sb = awk.tile([1, SC], F32, tag="rsb")
                    nc.scalar.activation(
                        r_sb[:], ps_ss[:], AF.Sqrt, bias=eps_sb[:], scale=1.0 / D
                    )
                    r16 = awk.tile([1, SC], F16, tag="r16")
                    nc.scalar.copy(r16[:], r_sb[:])
                    ps_rb = apsum.tile([64, SC], F32, tag="rb", bufs=1)
                    nc.tensor.matmul(
                        ps_rb[:], onesr_sb[0:1, 0:64], r16[:],
                        start=True, stop=True,
                    )
                    r64 = awk.tile([64, SC], F32, tag="r64")
                    nc.vector.reciprocal(r64[:], ps_rb[:])

                    # per-token scale for v: strided round-trip off the
                    # critical path (v matmuls run much later)
                    nc.sync.dma_start(r_dram[0:1, csl], r_sb[:])
                    r_tokr = awk.tile([128, NKBC], F32, tag="rtokr")
                    flat = r_dram[0:1, csl]
                    nc.sync.dma_start(
                        r_tokr[:],
                        bass.AP(tensor=flat.tensor, offset=flat.offset,
                                ap=[[1, 128], [128, NKBC]]),
                    )
                    r_tok = awk.tile([128, NKBC], F32, tag="rtok")
                    nc.vector.reciprocal(r_tok[:], r_tokr[:])

                    ctr = awk.tile([64, SC], F32, tag="ctr")
                    srt = awk.tile([64, SC], F32, tag="srt")
                    nc.vector.tensor_mul(ctr[:], ctab_sb[:, csl], r64[:])
                    nc.vector.tensor_mul(srt[:], stab_sb[:, csl], r64[:])

                    # q/k matmuls + rope (features de-interleaved on host:
                    # rotary-even dims on partitions 0:64, odd on 64:128)
                    def rope(ps_qk, m):
                        dst = q_sb if m < HC else k_sb
                        hh = m if m < HC else m - HC
                        t1 = awk.tile([64, SC], F32, tag="t1")
                        t2 = awk.tile([64, SC], F32, tag="t2")
                        nc.vector.tensor_mul(t1[:], ps_qk[0:64, :], ctr[:])
                        nc.vector.tensor_mul(t2[:], ps_qk[64:128, :], srt[:])
                        nc.vector.tensor_sub(dst[0:64, hh, csl], t1[:], t2[:])
                        nc.vector.tensor_mul(t1[:], ps_qk[64:128, :], ctr[:])
                        nc.vector.tensor_mul(t2[:], ps_qk[0:64, :], srt[:])
                        nc.vector.tensor_add(dst[64:128, hh, csl], t1[:], t2[:])

                    rope(ps_qk0, 0)
                    for m in range(1, NQK):
                        wm = astr.tile([128, ND, 128], F16, tag="wqk")
                        nc.sync.dma_start(wm[:], wqk_in[m])
                        if c == 0 and m == 1:
                            nc.sync.dma_start(wv_sb[:], wv_in[:])
                        ps_qk = apsum.tile([128, SC], F32, tag="qk")
                        for d in range(ND):
                            nc.tensor.matmul(
                                ps_qk[:], wm[:, d, :], x_sb[:, d, :],
                                start=(d == 0), stop=(d == ND - 1),
                            )
                        rope(ps_qk, m)

                    # v matmuls (token-major), fold rms scale into PSUM copy
                    for sb_i in range(NKBC):
                        ps_v = apsum.tile([128, NV], F32, tag="v", bufs=2)
                        for d in range(ND):
                            nc.tensor.matmul(
                                ps_v[:], x_sb[:, d, sb_i * 128:(sb_i + 1) * 128],
                                wv_sb[:, d, :],
                                start=(d == 0), stop=(d == ND - 1),
                            )
                        nc.scalar.activation(
                            v_sb[:, c * NKBC + sb_i, :], ps_v[:], AF.Copy,
                            scale=r_tok[:, sb_i:sb_i + 1],
                        )

            # ====== residual tile lives from phase C into phase D ======
            with tc.tile_pool(name="res", bufs=1) as resp:
                out1 = resp.tile([128, ND, SC], F32)
                r2h = resp.tile([1, SC], F16)

                with tc.tile_pool(name="wout", bufs=1) as woutp:
                    wout_sb = woutp.tile([128, ND, D], F16)

                    # ========== phase B: attention + per-chunk AllGather ==========
                    with (
                        tc.tile_pool(name="b_m", bufs=1) as bmp,
                        tc.tile_pool(name="b_work", bufs=2) as bwk,
                        tc.tile_pool(name="b_pt", bufs=8) as bpt,
                        tc.tile_pool(name="b_av", bufs=2) as bav,
                        tc.tile_pool(name="b_ps", bufs=3, space="PSUM") as bps,
                        tc.tile_pool(name="b_ps_acc", bufs=2, space="PSUM") as bpsa,
                    ):
                        masks = bmp.tile([128, NKBC, SC], F16)
                        nc.sync.dma_start(masks[:], mask_in[:])
                        nc.sync.dma_start(wout_sb[:], wout_in[:])

                        for c in range(4):
                            csl = slice(c * SC, (c + 1) * SC)
                            nkb = (c + 1) * NKBC
                            avf = bav.tile([128, HC, SC], F16, tag="avf")
                            for h in range(HC):
                                ps_av = bpsa.tile([128, SC], F32, tag="av")
                                ps_den = bpsa.tile([1, SC], F32, tag="dn", bufs=1)
                                for w0 in range(0, nkb, 4):
                                    wave = range(w0, min(w0 + 4, nkb))
                                    pts = {}
                                    for kb in wave:
                                        ps_sc = bps.tile([128, SC], F32, tag="sc")
                                        o = kb - c * NKBC
                                        if o >= 0:
                                            nc.tensor.matmul(
                                                ps_sc[:], ident_sb[:],
                                                masks[:, o, :],
                                                start=True, stop=False,
                                            )
                                        nc.tensor.matmul(
                                            ps_sc[:],
                                            k_sb[:, h, kb * 128:(kb + 1) * 128],
                                            q_sb[:, h, csl],
                                            start=(o < 0), stop=True,
                                        )
                                        pT = bpt.tile([128, SC], F16, tag="pt")
                                        nc.scalar.activation(
                                            pT[:], ps_sc[:], AF.Exp, scale=ISQ,
                                            bias=expb_sb[:],
                                        )
                                        pts[kb] = pT
                                    for kb in wave:
                                        nc.tensor.matmul(
                                            ps_av[:],
                                            v_sb[:, kb, h * DH:(h + 1) * DH],
                                            pts[kb][:],
                                            start=(kb == 0), stop=(kb == nkb - 1),
                                        )
                                        nc.tensor.matmul(
                                            ps_den[:], ones_sb[:], pts[kb][:],
                                            start=(kb == 0), stop=(kb == nkb - 1),
                                        )
                                # 1/den: fp16 copy -> PE ones-broadcast ->
                                # partition-parallel reciprocal (no DRAM trip)
                                den16 = bwk.tile([1, SC], F16, tag="den16")
                                nc.scalar.copy(den16[:], ps_den[:])
                                ps_db = bpsa.tile([128, SC], F32, tag="db")
                                nc.tensor.matmul(
                                    ps_db[:], onesr_sb[:], den16[:],
                                    start=True, stop=True,
                                )
                                denb = bwk.tile([128, SC], F32, tag="denb")
                                nc.vector.reciprocal(denb[:], ps_db[:])
                                nc.vector.tensor_mul(avf[:, h, :], ps_av[:], denb[:])
                                nc.sync.dma_start(ag_in[c][:, h, :], avf[:, h, :])
                            nc.gpsimd.collective_compute(
                                "AllGather",
                                mybir.AluOpType.bypass,
                                replica_groups=CGRP,
                                ins=[ag_in[c][:].opt()],
                                outs=[ag_all[c].opt()],
                            )

                    # ====== phase C: own-chunk out_proj + residual + stats ======
                    with (
                        tc.tile_pool(name="c_big", bufs=1) as cbp,
                        tc.tile_pool(name="c_work", bufs=2) as cwk,
                        tc.tile_pool(name="c_ps", bufs=2, space="PSUM") as cps,
                        tc.tile_pool(name="c_ps1", bufs=1, space="PSUM") as cps1,
                    ):
                        xr_sb = cbp.tile([128, ND, SC], F16)
                        nc.sync.dma_start(xr_sb[:], xr_in[:])
                        av_g = cbp.tile([128, ND, SC], F16)
                        tsel = nc.sync.partition_id() % G
# own chunk of the gathered buffer: [p][rank][head][s]
                        src = bass.AP(
                            tensor=ag_all.tensor,
                            offset=ag_all.offset + tsel * (G * 128 * HC * SC),
                            ap=[[HC * SC, 128], [128 * HC * SC, G],
                                [SC, HC], [1, SC]],
                        )
                        nc.sync.dma_start(av_g[:], src)

                        ps_ss2 = cps1.tile([1, SC], F32, tag="ss2")
                        xsqs = []
                        for m in range(ND):
                            ps_op = cps.tile([128, SC], F32, tag="op")
                            for fb in range(ND):
                                nc.tensor.matmul(
                                    ps_op[:],
                                    wout_sb[:, fb, m * 128:(m + 1) * 128],
                                    av_g[:, fb, :],
                                    start=(fb == 0), stop=(fb == ND - 1),
                                )
                            # stats matmul for the previous block: keeps the
                            # scalar-engine square off the PE critical path
                            if m > 0:
                                nc.tensor.matmul(
                                    ps_ss2[:], ones_sb[:], xsqs[m - 1][:],
                                    start=(m == 1), stop=False,
                                )
                            nc.vector.tensor_add(
                                out1[:, m, :], ps_op[:], xr_sb[:, m, :]
                            )
                            xsq2 = cwk.tile([128, SC], F16, tag="xsq2")
                            nc.scalar.square(xsq2[:], out1[:, m, :])
                            xsqs.append(xsq2)
                        nc.tensor.matmul(
                            ps_ss2[:], ones_sb[:], xsqs[ND - 1][:],
                            start=False, stop=True,
                        )
                        r2_sb = cwk.tile([1, SC], F32, tag="r2")
                        nc.scalar.activation(
                            r2_sb[:], ps_ss2[:], AF.Sqrt,
                            bias=eps_sb[:], scale=1.0 / D,
                        )
                        nc.scalar.copy(r2h[:], r2_sb[:])

                # ============ phase D: RMSNorm2 scale + MLP ==============
                with (
                    tc.tile_pool(name="d_big", bufs=1) as dbp,
                    tc.tile_pool(name="d_stream", bufs=3) as dstr,
                    tc.tile_pool(name="d_work", bufs=2) as dwk,
                    tc.tile_pool(name="d_ps", bufs=3, space="PSUM") as dps,
                ):
                    # broadcast rms2 across partitions via a ones outer
                    # product on the PE, then partition-parallel reciprocal
                    ps_r2b = dps.tile([128, SC], F32, tag="r2b", bufs=1)
                    nc.tensor.matmul(
                        ps_r2b[:], onesr_sb[:], r2h[:], start=True, stop=True
                    )
                    r2r = dbp.tile([128, SC], F32)
                    nc.vector.reciprocal(r2r[:], ps_r2b[:])
                    h2 = dbp.tile([128, ND, SC], F16)
                    for d in range(ND):
                        nc.vector.tensor_mul(h2[:, d, :], out1[:, d, :], r2r[:])

                    gT = dbp.tile([128, NFF, SC], F16)
                    for m in range(NFF):
                        wm = dstr.tile([128, ND, 128], F16, tag="wfc")
                        nc.sync.dma_start(wm[:], wfc_in[m])
                        ps_fc = dps.tile([128, SC], F32, tag="fc")
                        for d in range(ND):
                            nc.tensor.matmul(
                                ps_fc[:], wm[:, d, :], h2[:, d, :],
                                start=(d == 0), stop=(d == ND - 1),
                            )
                        nc.scalar.activation(gT[:, m, :], ps_fc[:], gelu)

                    for m in range(ND):
                        wm2 = dstr.tile([128, NFF, 128], F16, tag="wproj", bufs=2)
                        nc.sync.dma_start(wm2[:], wproj_in[m])
                        ps_pr = dps.tile([128, SC], F32, tag="pr")
                        for k in range(NFF):
                            nc.tensor.matmul(
                                ps_pr[:], wm2[:, k, :], gT[:, k, :],
                                start=(k == 0), stop=(k == NFF - 1),
                            )
                        o2 = dwk.tile([128, SC], F32, tag="o2")
                        nc.vector.tensor_add(o2[:], ps_pr[:], out1[:, m, :])
                        nc.sync.dma_start(out_ext[m], o2[:])

    nc.compile()
    return nc


def _deinterleave(w):
    """Reorder head-dim columns: evens then odds (per 128-wide head)."""
    Din, Dout = w.shape
    nh = Dout // DH
    w4 = w.reshape(Din, nh, DH // 2, 2)
    return np.concatenate([w4[..., 0], w4[..., 1]], axis=2).reshape(Din, Dout)


def prep_inputs(x, w_qkv, w_out, w_fc, w_proj, g_in, g_ff, S, D, H, FF):
    HC = H // G
    SC = S // 4
    ND = D // 128
    NQK = 2 * HC
    NV = HC * DH
    NKBC = SC // 128
    NFF = FF // 128

    x = np.asarray(x, np.float32)
    w_qkv = np.asarray(w_qkv, np.float32)
    w_out = np.asarray(w_out, np.float32)
    w_fc = np.asarray(w_fc, np.float32)
    w_proj = np.asarray(w_proj, np.float32)
    g_in = np.asarray(g_in, np.float32)
    g_ff = np.asarray(g_ff, np.float32)

    wq = w_qkv * g_in[:, None]

    half = DH // 2
    invf = 1.0 / (ROPE_BASE ** (2.0 * np.arange(half, dtype=np.float64) / DH))
    ang = np.arange(S, dtype=np.float64)[:, None] * invf[None, :]
    ctab = np.ascontiguousarray(np.cos(ang).T.astype(np.float32))
    stab = np.ascontiguousarray(np.sin(ang).T.astype(np.float32))

    masks = np.zeros((NKBC, 128, SC), np.float32)
    pp = np.arange(128)[:, None]
    qq = np.arange(SC)[None, :]
    for o in range(NKBC):
        masks[o] = np.where(o * 128 + pp <= qq, 0.0, -60000.0).astype(np.float32)
    masks = np.ascontiguousarray(masks.transpose(1, 0, 2)).astype(np.float16)
    ident = np.eye(128, dtype=np.float16)

    wfc = w_fc * g_ff[:, None]
    wfc_r = np.ascontiguousarray(
        wfc.reshape(ND, 128, NFF, 128).transpose(2, 1, 0, 3)
    ).astype(np.float16)
    wproj_r = np.ascontiguousarray(
        w_proj.reshape(NFF, 128, ND, 128).transpose(2, 1, 0, 3)
    ).astype(np.float16)
    wout_r = np.ascontiguousarray(
        w_out.reshape(ND, 128, D).transpose(1, 0, 2)
    ).astype(np.float16)
    ones = np.ones((128, 1), np.float16)
    onesr = np.ones((1, 128), np.float16)

    in_maps = []
    for core in range(NCORES):
        b, t = core // G, core % G
        xb = x[b]
        xT = np.ascontiguousarray(xb.T)
        xT_r = np.ascontiguousarray(
            xT.reshape(ND, 128, 4, SC).transpose(2, 1, 0, 3)
        ).astype(np.float16)
        xr = np.ascontiguousarray(
            xb[t * SC:(t + 1) * SC, :].T.reshape(ND, 128, SC).transpose(1, 0, 2)
        ).astype(np.float16)
        qcols = _deinterleave(wq[:, t * NV:(t + 1) * NV])
        kcols = _deinterleave(wq[:, D + t * NV:D + (t + 1) * NV])
        vcols = wq[:, 2 * D + t * NV:2 * D + (t + 1) * NV]
        wqk_core = np.ascontiguousarray(
            np.concatenate([qcols, kcols], axis=1)
            .reshape(ND, 128, NQK, 128).transpose(2, 1, 0, 3)
        ).astype(np.float16)
        wv_core = np.ascontiguousarray(
            vcols.reshape(ND, 128, NV).transpose(1, 0, 2)
        ).astype(np.float16)
        in_maps.append({
            "xT": xT_r, "xr": xr, "wqk": wqk_core, "wv": wv_core,
            "wout": wout_r, "wfc": wfc_r, "wproj": wproj_r,
            "ctab": ctab, "stab": stab, "masks": masks, "ones": ones,
            "onesr": onesr, "ident": ident,
        })
    return in_maps


def assemble(results, S, D):
    SC = S // 4
    y = np.zeros((2, S, D), np.float32)
    for core in range(NCORES):
        b, t = core // G, core % G
        o = results[core]["out"]
        y[b, t * SC:(t + 1) * SC, :] = o.reshape(D, SC).T
    return y


_CACHE = {}


def run(inputs, S, D, H, FF, trace=False, **kw):
    key = (S, D, H, FF)
    if key not in _CACHE:
        _CACHE[key] = build_nc(S, D, H, FF)
    nc = _CACHE[key]
    in_maps = prep_inputs(
        inputs["x"], inputs["w_qkv"], inputs["w_out"], inputs["w_fc"],
        inputs["w_proj"], inputs["g_in"], inputs["g_ff"], S, D, H, FF,
    )
    res = run_bass_kernel_spmd(nc, in_maps, list(range(NCORES)), trace=trace, **kw)
    return assemble(res.results, S, D), res


def kernel(**inputs):
    y, _ = run(inputs, S=2048, D=2048, H=16, FF=4096)
    return y.astype(np.float32)
